# revision 1
# baseline (speedup 1.0000x reference)
"""Trainium2 Bass kernel v2 for nn_EquivariantLayer (gnn_message_passing).

Design: per-node linear precompute T = [xs@Wss | xs@Wsv | xv_d@Wvv | xv_d@Wvs]
(384 cols, bf16) done ON DEVICE (replicated), so the per-edge tensor product
reduces to elementwise combinations with the 4 edge scalars.  Edges sharded by
destination core; per (window, group of 128 edges): indirect-gather T rows,
assemble messages with fused scalar_tensor_tensor ops, radial sigmoid gate,
one-hot scatter matmul accumulated in PSUM; per-window residual + o3.Linear.
"""
import numpy as np
import time
import jax
from jax.sharding import Mesh, PartitionSpec
from jax.experimental.shard_map import shard_map
import concourse.bass as bass
import concourse.mybir as mybir
import concourse.tile as tile
from concourse import bacc
from concourse.bass2jax import _bass_exec_p, install_neuronx_cc_hook, partition_id_tensor

F32 = mybir.dt.float32
BF16 = mybir.dt.bfloat16
I32 = mybir.dt.int32
BF16_NP = mybir.dt.np(BF16)
P = 128
N_NODES = 100000
N_CORES = 8
NPC = N_NODES // N_CORES          # 12500 nodes per core
NPADC = 12544                      # per-core node rows padded to 98*128
W_WIN = (NPC + P - 1) // P         # 98 windows per core
TROWS = N_CORES * NPADC            # 100352 rows in transformed table
NCHUNK = TROWS // P                # 784 transform chunks
MUL_S, MUL_V, DIM = 64, 32, 160
TD = 384                           # transformed feature width
ESC = 16
ISQ3 = 1.0 / np.sqrt(3.0)
ISQ2 = 1.0 / np.sqrt(2.0)

# planar permutation: planar col 64+32*d+u  <- interleaved col 64+3*u+d
PERM = np.concatenate(
    [np.arange(64)] + [64 + 3 * np.arange(32) + d for d in range(3)]
).astype(np.int64)

# T column blocks: [A(0:64) | C0C1C2(64:256) | D(256:288) | E0E1E2(288:384)]
A_OF, D_OF = 0, 256
def C_OF(d): return 64 + 64 * d
def E_OF(d): return 288 + 32 * d


def trow_of(node):
    return (node // NPC) * NPADC + (node % NPC)


def _balanced_windows(deg_core):
    """Assign len(deg_core) (<= W_WIN*P) nodes to W_WIN windows (<=128 each),
    balancing edge load.  Returns (lw, slot) arrays."""
    n = len(deg_core)
    order = np.argsort(-deg_core, kind="stable")
    i = np.arange(n)
    ws = i % (2 * W_WIN)
    ws = np.where(ws < W_WIN, ws, 2 * W_WIN - 1 - ws)
    lw = np.zeros(n, dtype=np.int64)
    lw[order] = ws
    loads = np.bincount(lw, weights=deg_core, minlength=W_WIN)
    # greedy repair toward max load <= 1024: swap nodes between hot/cold windows
    for _ in range(200):
        wmax = int(np.argmax(loads))
        if loads[wmax] <= 8 * P:
            break
        wmin = int(np.argmin(loads))
        need = loads[wmax] - 8 * P
        a_nodes = np.where(lw == wmax)[0]
        b_nodes = np.where(lw == wmin)[0]
        da, db = deg_core[a_nodes], deg_core[b_nodes]
        # pick pair with delta >= need, minimal overshoot
        diff = da[:, None] - db[None, :]
        cand = np.where(diff >= need, diff, np.iinfo(np.int64).max)
        ai, bi = np.unravel_index(np.argmin(cand), cand.shape)
        if cand[ai, bi] == np.iinfo(np.int64).max:
            ai, bi = np.unravel_index(np.argmax(diff), diff.shape)
            if diff[ai, bi] <= 0:
                break
        a, b = a_nodes[ai], b_nodes[bi]
        lw[a], lw[b] = wmin, wmax
        delta = deg_core[a] - deg_core[b]
        loads[wmax] -= delta
        loads[wmin] += delta
    # slots: stable position within window
    slot = np.zeros(n, dtype=np.int64)
    o = np.argsort(lw, kind="stable")
    pos = np.arange(n) - np.searchsorted(lw[o], lw[o])
    slot[o] = pos
    return lw, slot


def build_plan(edge_index, balance=True):
    src = np.asarray(edge_index[0], dtype=np.int64)
    dst = np.asarray(edge_index[1], dtype=np.int64)
    CAP = W_WIN * 8 * P              # per-core edge capacity at G=8
    if balance:
        deg = np.bincount(dst, minlength=N_NODES).astype(np.int64)
        core_of_node = np.arange(N_NODES) // NPC
        totals = np.bincount(core_of_node, weights=deg, minlength=N_CORES)
        ncount = np.bincount(core_of_node, minlength=N_CORES)
        # shed load from over-capacity cores by moving single nodes
        for _ in range(2000):
            cmax = int(np.argmax(totals))
            if totals[cmax] <= CAP - 8:
                break
            cmin = int(np.argmin(totals + np.where(ncount >= W_WIN * P, 1 << 40, 0)))
            cand = np.where(core_of_node == cmax)[0]
            mv = cand[np.argmax(deg[cand])]
            core_of_node[mv] = cmin
            totals[cmax] -= deg[mv]
            totals[cmin] += deg[mv]
            ncount[cmax] -= 1
            ncount[cmin] += 1
        lw_of = np.zeros(N_NODES, dtype=np.int64)
        slot_of = np.zeros(N_NODES, dtype=np.int64)
        nodes_of_core = []
        for c in range(N_CORES):
            nodes = np.where(core_of_node == c)[0]
            nodes_of_core.append(nodes)
            lwc, slc = _balanced_windows(deg[nodes])
            lw_of[nodes] = lwc
            slot_of[nodes] = slc
        core_of = core_of_node[dst]
        lw = lw_of[dst]
        slot = slot_of[dst]
    else:
        core_of_node = np.arange(N_NODES) // NPC
        nodes_of_core = [np.arange(c * NPC, (c + 1) * NPC) for c in range(N_CORES)]
        lw_of = (np.arange(N_NODES) % NPC) // P
        slot_of = (np.arange(N_NODES) % NPC) % P
        core_of = dst // NPC
        lw = lw_of[dst]
        slot = slot_of[dst]
    cw = core_of * W_WIN + lw
    counts = np.bincount(cw, minlength=N_CORES * W_WIN).reshape(N_CORES, W_WIN)
    G = int(np.ceil(counts.max() / P))
    order = np.argsort(cw, kind="stable")
    return dict(src=src, dst=dst, order=order, counts=counts, G=G,
                slot=slot, core_of=core_of, lw_of=lw_of, slot_of=slot_of,
                nodes_of_core=nodes_of_core, balance=balance)


def build_core_arrays(plan, c, edge_feat, edge_scalars):
    G = plan["G"]
    src, order, counts = plan["src"], plan["order"], plan["counts"]
    slot_all = plan["slot"]
    idx = np.zeros((W_WIN, P, G), dtype=np.int32)
    meta = np.zeros((W_WIN, P, 5 * G), dtype=np.float32)
    escT = np.zeros((W_WIN, 17, G * P), dtype=np.float32)
    escT[:, 16, :] = 1.0
    core_edges = order[(plan["core_of"][order] == c)]
    off = 0
    for w in range(W_WIN):
        n_e = counts[c, w]
        e_ids = core_edges[off:off + n_e]
        off += n_e
        j = np.arange(n_e)
        g, p = j // P, j % P
        idx[w, p, g] = trow_of(src[e_ids])
        meta[w, p, 5 * g + 0] = slot_all[e_ids]
        meta[w, p, 5 * g + 1] = edge_feat[e_ids, 0]
        for d in range(3):
            meta[w, p, 5 * g + 2 + d] = edge_feat[e_ids, 1 + d]
        escT[w, :16, g * P + p] = edge_scalars[e_ids]
    return dict(idx=idx, meta=meta, escT=escT.astype(BF16_NP))


def prep_weights(W_ss, W_sv, W_vs, W_vv, W_rad, b_rad, L_s, L_v):
    c_ss = ISQ2 / np.sqrt(MUL_S)
    c_vv = ISQ2 * ISQ3 / np.sqrt(MUL_V)
    c_sv = ISQ2 * ISQ3 / np.sqrt(MUL_S)
    c_vs = ISQ2 * ISQ3 / np.sqrt(MUL_V)
    # stationary chunk = [xs(0:64) | xv0(64:96) | xv1(96:128)]; second = xv2
    # w1 streams [A|C0|C1] (T cols 0:192); w1b streams [D|E0|E1] (cols 256:352)
    wbig = np.zeros((128, 192), dtype=np.float32)
    wbig[0:64, 0:64] = W_ss * c_ss
    wbig[64:96, 64:128] = W_vv * c_vv
    wbig[96:128, 128:192] = W_vv * c_vv
    w1b = np.zeros((128, 96), dtype=np.float32)
    w1b[0:64, 0:32] = W_sv * c_sv
    w1b[64:96, 32:64] = W_vs * c_vs
    w1b[96:128, 64:96] = W_vs * c_vs
    Wrs = np.zeros((17, DIM), dtype=np.float32)
    Wrs[:16] = W_rad.T[:, PERM]
    Wrs[16] = b_rad[PERM]
    Ls = (L_s / np.sqrt(MUL_S)).astype(np.float32)
    Lvrep = np.tile((L_v / np.sqrt(MUL_V)).astype(np.float32), (3, 1))  # [96,32]
    return dict(wbig=wbig.astype(BF16_NP), w1b=w1b.astype(BF16_NP),
                w2c=(W_vv * c_vv).astype(BF16_NP), w2e=(W_vs * c_vs).astype(BF16_NP),
                wrs=Wrs.astype(BF16_NP), ls=Ls.astype(BF16_NP),
                lvrep=Lvrep.astype(BF16_NP))


def host_transform(xp):
    """Reference host computation of T (padded rows), fp32 -> bf16."""
    T = np.zeros((TROWS, TD), dtype=np.float32)
    return T  # filled by caller with weights; placeholder (see kernel())


def make_xT(xp):
    """xT [160, TROWS] bf16, planar features x padded-row nodes."""
    xT = np.zeros((160, TROWS), dtype=BF16_NP)
    xpT = xp.T.astype(BF16_NP)
    for c in range(N_CORES):
        xT[:, c * NPADC:c * NPADC + NPC] = xpT[:, c * NPC:(c + 1) * NPC]
    return xT


def make_xw(xp, c, plan):
    xw = np.zeros((W_WIN * P, DIM), dtype=np.float32)
    nodes = plan["nodes_of_core"][c]
    fi = plan["lw_of"][nodes] * P + plan["slot_of"][nodes]
    xw[fi] = xp[nodes]
    return xw.reshape(W_WIN, P, DIM).astype(BF16_NP)


def build_nc_tf(CB=8):
    """Transform kernel: x^T -> Td table (replicated full table per core)."""
    nc = bacc.Bacc(None, target_bir_lowering=False)
    ncols = NPADC if agtf else TROWS
    NB = (ncols // P) // CB
    assert (ncols // P) % CB == 0
    xT = nc.declare_dram_parameter("xT", [160, ncols], BF16, isOutput=False)
    wbig = nc.declare_dram_parameter("wbig", [128, 288], BF16, isOutput=False)
    w2 = nc.declare_dram_parameter("w2", [32, 96], BF16, isOutput=False)
    Td = nc.declare_dram_parameter("Td", [TROWS, TD], BF16, isOutput=True)
    with tile.TileContext(nc) as tc:
        _emit_transform(nc, tc, xT, wbig, w2, Td, CB, NB)
    nc.compile()
    return nc


def _emit_transform(nc, tc, xT, wbig, w2, Td, CB, NB):
    with (
        tc.tile_pool(name="const_tf", bufs=1) as cpool,
        tc.tile_pool(name="tf", bufs=3) as tfpool,
        tc.tile_pool(name="tfst", bufs=10) as stpool,
        tc.tile_pool(name="tfps", bufs=3, space="PSUM") as tfps,
    ):
        c_wbig = cpool.tile([128, 288], BF16, tag="wbig")
        c_w2 = cpool.tile([32, 96], BF16, tag="w2")
        nc.sync.dma_start(out=c_wbig[:], in_=wbig[:])
        nc.sync.dma_start(out=c_w2[:], in_=w2[:])
        for b in range(NB):
            t_xT = tfpool.tile([128, CB * P], BF16, tag="xT")
            t_xT2 = tfpool.tile([32, CB * P], BF16, tag="xT2")
            col0 = b * CB * P
            nc.sync.dma_start(out=t_xT[:], in_=xT[0:128, col0:col0 + CB * P])
            nc.sync.dma_start(out=t_xT2[:], in_=xT[128:160, col0:col0 + CB * P])
            for k in range(CB):
                p_t = tfps.tile([128, TD], F32, tag="pt", space="PSUM")
                nc.tensor.matmul(out=p_t[:, 0:288],
                                 lhsT=t_xT[:, k * P:(k + 1) * P],
                                 rhs=c_wbig[:], start=True, stop=True)
                nc.tensor.matmul(out=p_t[:, 288:384],
                                 lhsT=t_xT2[:, k * P:(k + 1) * P],
                                 rhs=c_w2[:], start=True, stop=True)
                t_tc = stpool.tile([128, TD], BF16, tag="tc")
                if (b * CB + k) % 2 == 0:
                    nc.vector.tensor_copy(out=t_tc[:], in_=p_t[:])
                else:
                    nc.scalar.copy(out=t_tc[:], in_=p_t[:])
                r0 = (b * CB + k) * P
                nc.sync.dma_start(out=Td[r0:r0 + P, :], in_=t_tc[:])


def _assemble(nc, t_msg, t_xe, t_meta, t_dv_pool, g, xb, es):
    nc.scalar.mul(t_msg[:, 0:64], t_xe[:, xb + A_OF:xb + A_OF + 64], es)
    for d in range(3):
        evd = t_meta[:, 5 * g + 2 + d:5 * g + 3 + d]
        nc.vector.scalar_tensor_tensor(
            out=t_msg[:, 0:64],
            in0=t_xe[:, xb + C_OF(d):xb + C_OF(d) + 64],
            scalar=evd, in1=t_msg[:, 0:64],
            op0=mybir.AluOpType.mult, op1=mybir.AluOpType.add)
    t_dv = t_dv_pool.tile([P, 96], BF16, tag="dv")
    for d in range(3):
        evd = t_meta[:, 5 * g + 2 + d:5 * g + 3 + d]
        nc.scalar.mul(t_dv[:, 32 * d:32 * d + 32],
                      t_xe[:, xb + D_OF:xb + D_OF + 32], evd)
    # E block is contiguous: one fused (E*es + dv) over all 96 v-cols
    nc.vector.scalar_tensor_tensor(
        out=t_msg[:, 64:160],
        in0=t_xe[:, xb + E_OF(0):xb + E_OF(0) + 96],
        scalar=es, in1=t_dv[:, 0:96],
        op0=mybir.AluOpType.mult, op1=mybir.AluOpType.add)


def build_nc_main(G, skip=()):
    """Main kernel: gather Td rows, assemble gated messages, scatter, linear."""
    nc = bacc.Bacc(None, target_bir_lowering=False)
    EW = G * P
    Td = nc.declare_dram_parameter("Td", [TROWS, TD], BF16, isOutput=False)
    prm = _declare_main_params(nc, G)
    with tile.TileContext(nc) as tc:
        with (
            tc.tile_pool(name="const", bufs=1) as cpool,
            tc.tile_pool(name="win", bufs=3) as wpool,
            tc.tile_pool(name="grp", bufs=4) as gpool,
            tc.tile_pool(name="psr", bufs=2, space="PSUM") as psr,
            tc.tile_pool(name="psagg", bufs=2, space="PSUM") as psagg,
            tc.tile_pool(name="psht", bufs=1, space="PSUM") as psht,
            tc.tile_pool(name="psy", bufs=1, space="PSUM") as psy,
        ):
            pools = dict(cpool=cpool, wpool=wpool, gpool=gpool, psr=psr,
                         psagg=psagg, psht=psht, psy=psy)
            _emit_main(nc, tc, Td, prm, pools, G, skip)
    nc.compile()
    return nc


def build_nc_merged(G, CB=8, skip=(), agtf=False):
    """Single program: transform phase, all-engine barrier, then main phase."""
    nc = bacc.Bacc(None, target_bir_lowering=False)
    ncols = NPADC if agtf else TROWS
    NB = (ncols // P) // CB
    assert (ncols // P) % CB == 0
    xT = nc.declare_dram_parameter("xT", [160, ncols], BF16, isOutput=False)
    wbig = nc.declare_dram_parameter("wbig", [128, 192], BF16, isOutput=False)
    w1b = nc.declare_dram_parameter("w1b", [128, 96], BF16, isOutput=False)
    w2c = nc.declare_dram_parameter("w2c", [32, 64], BF16, isOutput=False)
    w2e = nc.declare_dram_parameter("w2e", [32, 32], BF16, isOutput=False)
    prm = _declare_main_params(nc, G)
    Td = nc.dram_tensor("Td", [TROWS, TD], BF16)
    Td_slice = nc.dram_tensor("Td_slice", [NPADC, TD], BF16) if agtf else Td
    if "tf" in skip:
        NB = 1
    with tile.TileContext(nc) as tc:
        with (
            tc.tile_pool(name="const_tf", bufs=1) as cpool_tf,
            tc.tile_pool(name="tf", bufs=3) as tfpool,
            tc.tile_pool(name="tfst", bufs=10) as stpool,
            tc.tile_pool(name="tfps", bufs=3, space="PSUM") as tfps,
            tc.tile_pool(name="const", bufs=1) as cpool,
            tc.tile_pool(name="win", bufs=3) as wpool,
            tc.tile_pool(name="grp", bufs=8) as gpool,
            tc.tile_pool(name="psr", bufs=2, space="PSUM") as psr,
            tc.tile_pool(name="psagg", bufs=2, space="PSUM") as psagg,
            tc.tile_pool(name="psht", bufs=1, space="PSUM") as psht,
        ):
            c_wbig = cpool_tf.tile([128, 192], BF16, tag="wbig")
            c_w1b = cpool_tf.tile([128, 96], BF16, tag="w1b")
            c_w2c = cpool_tf.tile([32, 64], BF16, tag="w2c")
            c_w2e = cpool_tf.tile([32, 32], BF16, tag="w2e")
            nc.sync.dma_start(out=c_wbig[:], in_=wbig[:])
            nc.sync.dma_start(out=c_w1b[:], in_=w1b[:])
            nc.sync.dma_start(out=c_w2c[:], in_=w2c[:])
            nc.sync.dma_start(out=c_w2e[:], in_=w2e[:])
            for b in range(NB):
                t_xT = tfpool.tile([128, CB * P], BF16, tag="xT")
                t_xT2 = tfpool.tile([32, CB * P], BF16, tag="xT2")
                col0 = b * CB * P
                nc.sync.dma_start(out=t_xT[:], in_=xT[0:128, col0:col0 + CB * P])
                nc.sync.dma_start(out=t_xT2[:], in_=xT[128:160, col0:col0 + CB * P])
                for k in range(CB):
                    p_t = tfps.tile([128, TD], F32, tag="pt", space="PSUM")
                    nc.tensor.matmul(out=p_t[:, 0:192],
                                     lhsT=t_xT[:, k * P:(k + 1) * P],
                                     rhs=c_wbig[:], start=True, stop=True)
                    nc.tensor.matmul(out=p_t[:, 256:352],
                                     lhsT=t_xT[:, k * P:(k + 1) * P],
                                     rhs=c_w1b[:], start=True, stop=True)
                    nc.tensor.matmul(out=p_t[:, 192:256],
                                     lhsT=t_xT2[:, k * P:(k + 1) * P],
                                     rhs=c_w2c[:], start=True, stop=True)
                    nc.tensor.matmul(out=p_t[:, 352:384],
                                     lhsT=t_xT2[:, k * P:(k + 1) * P],
                                     rhs=c_w2e[:], start=True, stop=True)
                    t_tc = stpool.tile([128, TD], BF16, tag="tc")
                    if (b * CB + k) % 2 == 0:
                        nc.vector.tensor_copy(out=t_tc[:], in_=p_t[:])
                    else:
                        nc.scalar.copy(out=t_tc[:], in_=p_t[:])
                    r0 = (b * CB + k) * P
                    nc.sync.dma_start(out=Td_slice[r0:r0 + P, :], in_=t_tc[:])

            tc.strict_bb_all_engine_barrier()
            if agtf:
                nc.gpsimd.collective_compute(
                    kind="AllGather", op=mybir.AluOpType.bypass,
                    replica_groups=[[0, 1, 2, 3, 4, 5, 6, 7]],
                    ins=[Td_slice[:, :]], outs=[Td[:, :]])

            pools = dict(cpool=cpool, wpool=wpool, gpool=gpool, psr=psr,
                         psagg=psagg, psht=psht, psy=psr)
            if "main" in skip:
                t_y0 = wpool.tile([P, DIM], F32, tag="y0")
                nc.vector.memset(t_y0[:], 0.0)
                for w in range(W_WIN):
                    nc.sync.dma_start(out=prm["Y"][w], in_=t_y0[:])
            else:
                _emit_main(nc, tc, Td, prm, pools, G, skip)
    nc.compile()
    return nc


def _declare_main_params(nc, G):
    EW = G * P
    d = {}
    d["idx"] = nc.declare_dram_parameter("idx", [W_WIN, P, G], I32, isOutput=False)
    d["meta"] = nc.declare_dram_parameter("meta", [W_WIN, P, 5 * G], F32, isOutput=False)
    d["escT"] = nc.declare_dram_parameter("escT", [W_WIN, 17, EW], BF16, isOutput=False)
    d["xw"] = nc.declare_dram_parameter("xw", [W_WIN, P, DIM], BF16, isOutput=False)
    d["wrs"] = nc.declare_dram_parameter("wrs", [17, DIM], BF16, isOutput=False)
    d["ls"] = nc.declare_dram_parameter("ls", [64, 64], BF16, isOutput=False)
    d["lvrep"] = nc.declare_dram_parameter("lvrep", [96, 32], BF16, isOutput=False)
    d["iota"] = nc.declare_dram_parameter("iota", [P, P], BF16, isOutput=False)
    d["ident"] = nc.declare_dram_parameter("ident", [P, P], BF16, isOutput=False)
    d["Y"] = nc.declare_dram_parameter("y", [W_WIN, P, DIM], F32, isOutput=True)
    return d


def _emit_main(nc, tc, Td, prm, pools, G, skip=()):
    EW = G * P
    idx, meta, escT, xw = prm["idx"], prm["meta"], prm["escT"], prm["xw"]
    wrs, ls, lvrep, iota, ident, Y = (prm["wrs"], prm["ls"], prm["lvrep"],
                                      prm["iota"], prm["ident"], prm["Y"])
    cpool, wpool, gpool = pools["cpool"], pools["wpool"], pools["gpool"]
    psr, psagg, psht, psy = pools["psr"], pools["psagg"], pools["psht"], pools["psy"]
    c_wrs = cpool.tile([17, DIM], BF16, tag="wrs")
    c_ls = cpool.tile([64, 64], BF16, tag="ls")
    c_lvrep = cpool.tile([96, 32], BF16, tag="lvrep")
    c_iota = cpool.tile([P, P], BF16, tag="iota")
    c_id = cpool.tile([P, P], BF16, tag="ident")
    for t, d_ in ((c_wrs, wrs), (c_ls, ls),
                  (c_lvrep, lvrep), (c_iota, iota), (c_id, ident)):
        nc.sync.dma_start(out=t[:], in_=d_[:])

    for w in range(W_WIN):
        t_idx = wpool.tile([P, G], I32, tag="idx")
        t_meta = wpool.tile([P, 5 * G], F32, tag="meta")
        t_escT = wpool.tile([17, EW], BF16, tag="escT")
        t_xw = wpool.tile([P, DIM], BF16, tag="xw")
        nc.sync.dma_start(out=t_idx[:], in_=idx[w])
        nc.sync.dma_start(out=t_meta[:], in_=meta[w])
        nc.sync.dma_start(out=t_escT[:], in_=escT[w])
        nc.sync.dma_start(out=t_xw[:], in_=xw[w])
        t_xe = wpool.tile([P, G * TD], BF16, tag="xe")
        if "gather" in skip:
            nc.vector.memset(t_xe[:], 0.0)
        else:
            for g in range(G):
                nc.gpsimd.indirect_dma_start(
                    out=t_xe[:, g * TD:(g + 1) * TD], out_offset=None,
                    in_=Td[:, :],
                    in_offset=bass.IndirectOffsetOnAxis(
                        ap=t_idx[:, g:g + 1], axis=0))

        p_agg = psagg.tile([P, DIM], F32, tag="agg", space="PSUM")
        for g in range(G):
            xb = g * TD
            dstw = t_meta[:, 5 * g + 0:5 * g + 1]
            es = t_meta[:, 5 * g + 1:5 * g + 2]
            # radial gate
            t_scale = gpool.tile([P, DIM], BF16, tag="scale")
            if "radial" in skip:
                nc.vector.memset(t_scale[:], 0.5)
            else:
                p_r = psr.tile([P, DIM], F32, tag="pr", space="PSUM")
                nc.tensor.matmul(out=p_r[:], lhsT=t_escT[:, g * P:(g + 1) * P],
                                 rhs=c_wrs[:], start=True, stop=True)
                nc.scalar.activation(out=t_scale[:], in_=p_r[:],
                                     func=mybir.ActivationFunctionType.Sigmoid)
            # message assembly
            t_msg = gpool.tile([P, DIM], BF16, tag="msg")
            if "assembly" in skip:
                nc.vector.memset(t_msg[:], 1.0)
            else:
                _assemble(nc, t_msg, t_xe, t_meta, t_dv_pool=gpool, g=g, xb=xb, es=es)
            # gate + one-hot scatter (gpsimd reserved for gathers)
            t_msgg = gpool.tile([P, DIM], BF16, tag="msgg")
            nc.vector.tensor_tensor(out=t_msgg[:], in0=t_msg[:],
                                    in1=t_scale[:], op=mybir.AluOpType.mult)
            t_S = gpool.tile([P, P], BF16, tag="S")
            nc.vector.tensor_scalar(out=t_S[:], in0=c_iota[:],
                                    scalar1=dstw, scalar2=None,
                                    op0=mybir.AluOpType.is_equal)
            if "accum" in skip:
                nc.tensor.matmul(out=p_agg[:], lhsT=t_S[:], rhs=t_msgg[:],
                                 start=True, stop=True)
            elif "scatter" not in skip:
                nc.tensor.matmul(out=p_agg[:], lhsT=t_S[:], rhs=t_msgg[:],
                                 start=(g == 0), stop=(g == G - 1))

        # epilogue: h = xw + agg; y = h @ L per irrep
        if "scatter" in skip:
            nc.tensor.matmul(out=p_agg[:], lhsT=c_iota[:], rhs=t_xw[:],
                             start=True, stop=True)
        if "epilogue" in skip:
            t_y = wpool.tile([P, DIM], F32, tag="y")
            nc.vector.tensor_copy(out=t_y[:], in_=p_agg[:])
            nc.sync.dma_start(out=Y[w], in_=t_y[:])
            continue
        t_h = wpool.tile([P, DIM], BF16, tag="h")
        nc.vector.tensor_tensor(out=t_h[:], in0=p_agg[:], in1=t_xw[:],
                                op=mybir.AluOpType.add)
        # baseline-style transposed layout: blocks side by side in cols
        p_hT = psht.tile([64, 512], BF16, tag="hT", space="PSUM")
        nc.tensor.transpose(out=p_hT[:, 0:128], in_=t_h[:, 0:64],
                            identity=c_id[:])
        for d in range(3):
            nc.tensor.transpose(out=p_hT[0:32, 128 + 128 * d:256 + 128 * d],
                                in_=t_h[:, 64 + 32 * d:96 + 32 * d],
                                identity=c_id[:])
        t_hT = wpool.tile([64, 512], BF16, tag="hTs")
        nc.vector.tensor_copy(out=t_hT[:, 0:128], in_=p_hT[:, 0:128])
        nc.vector.tensor_copy(out=t_hT[0:32, 128:512], in_=p_hT[0:32, 128:512])
        p_y = psy.tile([P, DIM], F32, tag="pr", space="PSUM")
        nc.tensor.matmul(out=p_y[:, 0:64], lhsT=t_hT[:, 0:128],
                         rhs=c_ls[:], start=True, stop=True)
        for d in range(3):
            nc.tensor.matmul(out=p_y[:, 64 + 32 * d:96 + 32 * d],
                             lhsT=t_hT[0:32, 128 + 128 * d:256 + 128 * d],
                             rhs=c_lvrep[0:32, :],
                             start=True, stop=True)
        t_y = wpool.tile([P, DIM], F32, tag="y")
        nc.vector.tensor_copy(out=t_y[:], in_=p_y[:])
        nc.sync.dma_start(out=Y[w], in_=t_y[:])


class SpmdRunner:
    def __init__(self, nc, n_cores=8):
        install_neuronx_cc_hook()
        self.nc = nc
        self.n_cores = n_cores
        assert nc.dbg_addr is None or not nc.dbg_callbacks
        partition_name = nc.partition_id_tensor.name if nc.partition_id_tensor else None
        in_names, out_names, out_avals, zero_outs = [], [], [], []
        for alloc in nc.m.functions[0].allocations:
            if not isinstance(alloc, mybir.MemoryLocationSet):
                continue
            name = alloc.memorylocations[0].name
            if alloc.kind == "ExternalInput":
                if name != partition_name:
                    in_names.append(name)
            elif alloc.kind == "ExternalOutput":
                shape = tuple(alloc.tensor_shape)
                dtype = mybir.dt.np(alloc.dtype)
                out_names.append(name)
                out_avals.append(jax.core.ShapedArray(shape, dtype))
                zero_outs.append(np.zeros(shape, dtype))
        self.in_names, self.out_names = in_names, out_names
        self.out_avals, self.zero_outs = out_avals, zero_outs
        n_params, n_outs = len(in_names), len(out_names)
        self.n_params = n_params
        all_in_names = list(in_names) + list(out_names)
        if partition_name is not None:
            all_in_names.append(partition_name)

        def _body(*args):
            operands = list(args)
            if partition_name is not None:
                operands.append(partition_id_tensor())
            outs = _bass_exec_p.bind(
                *operands,
                out_avals=tuple(out_avals),
                in_names=tuple(all_in_names),
                out_names=tuple(out_names),
                lowering_input_output_aliases=(),
                sim_require_finite=False,
                sim_require_nnan=False,
                nc=nc,
            )
            return tuple(outs)

        devices = jax.devices()[:n_cores]
        self.mesh = Mesh(np.asarray(devices), ("core",))
        in_specs = (PartitionSpec("core"),) * (n_params + n_outs)
        out_specs = (PartitionSpec("core"),) * n_outs
        self.fn = jax.jit(
            shard_map(_body, mesh=self.mesh, in_specs=in_specs, out_specs=out_specs,
                      check_rep=False),
            keep_unused=True,
        )

    def prepare(self, in_maps):
        """in_maps: per-core dicts; a value may also be a pre-sharded jax.Array
        (same for all cores, passed under the name in in_maps[0] only)."""
        sh = jax.sharding.NamedSharding(self.mesh, PartitionSpec("core"))
        args = []
        for i, n in enumerate(self.in_names):
            v0 = in_maps[0][n]
            if isinstance(v0, jax.Array):
                args.append(v0)
            else:
                args.append(jax.device_put(
                    np.concatenate([np.asarray(in_maps[c][n])
                                    for c in range(self.n_cores)], axis=0), sh))
        for z in self.zero_outs:
            args.append(jax.device_put(
                np.zeros((self.n_cores * z.shape[0], *z.shape[1:]), z.dtype), sh))
        self._args = args
        return self._args

    def run(self):
        outs = self.fn(*self._args)
        jax.block_until_ready(outs)
        return outs

    def results(self, outs):
        res = []
        for c in range(self.n_cores):
            d = {}
            for i, name in enumerate(self.out_names):
                d[name] = np.asarray(outs[i]).reshape(self.n_cores, *self.out_avals[i].shape)[c]
            res.append(d)
        return res

    def time(self, iters=10, warmup=2):
        for _ in range(warmup):
            self.run()
        ts = []
        for _ in range(iters):
            t0 = time.perf_counter()
            self.run()
            ts.append(time.perf_counter() - t0)
        return np.array(ts)

    def run_n(self, n):
        outs = [self.fn(*self._args) for _ in range(n)]
        jax.block_until_ready(outs)

    def time_slope(self, reps=9, iters=8, warmup=2):
        """Per-iteration device time via chained-dispatch slope: the RPC floor
        cancels in wall(reps) - wall(1)."""
        for _ in range(warmup):
            self.run_n(2)
        t1, tr = [], []
        for _ in range(iters):
            t0 = time.perf_counter()
            self.run_n(1)
            t1.append(time.perf_counter() - t0)
            t0 = time.perf_counter()
            self.run_n(reps)
            tr.append(time.perf_counter() - t0)
        t1m, trm = np.min(t1), np.min(tr)
        return (trm - t1m) / (reps - 1), t1m, trm


def assemble_output(y_cores, plan):
    out = np.zeros((N_NODES, DIM), dtype=np.float32)
    inv = np.argsort(PERM)
    for c, yc in enumerate(y_cores):
        flat = yc.reshape(W_WIN * P, DIM)
        nodes = plan["nodes_of_core"][c]
        fi = plan["lw_of"][nodes] * P + plan["slot_of"][nodes]
        out[nodes] = flat[fi][:, inv]
    return out


_CACHE = {}


class Pipeline:
    """Two-stage chained execution: transform kernel then main kernel."""

    def __init__(self, r_tf, r_main):
        self.r_tf = r_tf
        self.r_main = r_main

    def run(self):
        o1 = self.r_tf.fn(*self.r_tf._args)
        o2 = self.r_main.fn(*self.r_main._args)
        jax.block_until_ready((o1, o2))
        return o1, o2

    def run_fresh(self):
        """Correctness path: main consumes this call's transform output."""
        o1 = self.r_tf.fn(*self.r_tf._args)
        td_i = self.r_tf.out_names.index("Td")
        self.r_main._args[self.r_main.in_names.index("Td")] = o1[td_i]
        o2 = self.r_main.fn(*self.r_main._args)
        jax.block_until_ready(o2)
        return o2

    def time(self, iters=10, warmup=2):
        for _ in range(warmup):
            self.run()
        ts = []
        for _ in range(iters):
            t0 = time.perf_counter()
            self.run()
            ts.append(time.perf_counter() - t0)
        return np.array(ts)

    def run_n(self, n):
        outs = []
        for _ in range(n):
            outs.append(self.r_tf.fn(*self.r_tf._args))
            outs.append(self.r_main.fn(*self.r_main._args))
        jax.block_until_ready(outs)

    def time_slope(self, reps=9, iters=8, warmup=2):
        """Per-iteration device time via chained-dispatch slope: the RPC floor
        cancels in wall(reps) - wall(1)."""
        for _ in range(warmup):
            self.run_n(2)
        t1, tr = [], []
        for _ in range(iters):
            t0 = time.perf_counter()
            self.run_n(1)
            t1.append(time.perf_counter() - t0)
            t0 = time.perf_counter()
            self.run_n(reps)
            tr.append(time.perf_counter() - t0)
        t1m, trm = np.min(t1), np.min(tr)
        return (trm - t1m) / (reps - 1), t1m, trm


def kernel(x, edge_index, edge_feat, edge_scalars,
           W_ss, W_sv, W_vs, W_vv, W_rad, b_rad, L_s, L_v):
    x = np.asarray(x, dtype=np.float32)
    edge_index = np.asarray(edge_index)
    edge_feat = np.asarray(edge_feat, dtype=np.float32)
    edge_scalars = np.asarray(edge_scalars, dtype=np.float32)

    xp = np.ascontiguousarray(x[:, PERM])
    plan = build_plan(edge_index)
    G = plan["G"]
    wts = prep_weights(np.asarray(W_ss), np.asarray(W_sv), np.asarray(W_vs),
                       np.asarray(W_vv), np.asarray(W_rad), np.asarray(b_rad),
                       np.asarray(L_s), np.asarray(L_v))
    xT = make_xT(xp)
    key = ("merged-ag", G)
    if key not in _CACHE:
        _CACHE[key] = SpmdRunner(build_nc_merged(G, CB=7, agtf=True),
                                 n_cores=N_CORES)
    runner = _CACHE[key]
    in_maps = []
    for c in range(N_CORES):
        ca = build_core_arrays(plan, c, edge_feat, edge_scalars)
        in_maps.append(dict(
            xT=np.ascontiguousarray(xT[:, c * NPADC:(c + 1) * NPADC]),
            wbig=wts["wbig"], w1b=wts["w1b"], w2c=wts["w2c"],
            w2e=wts["w2e"],
            idx=ca["idx"], meta=ca["meta"], escT=ca["escT"],
            xw=make_xw(xp, c, plan),
            wrs=wts["wrs"], ls=wts["ls"], lvrep=wts["lvrep"],
            iota=np.tile(np.arange(P, dtype=np.float32)[None, :],
                         (P, 1)).astype(BF16_NP),
            ident=np.eye(P, dtype=np.float32).astype(BF16_NP)))
    runner.prepare(in_maps)
    outs = runner.run()
    res = runner.results(outs)
    return assemble_output([res[c]["y"] for c in range(N_CORES)], plan).astype(np.float32)

